# revision 12
# baseline (speedup 1.0000x reference)
"""GCN graph encoder on 8 Trainium2 NeuronCores (Bass/Tile SPMD).

Sharding: nodes (and incident edges, by dst) split 8 ways. Per GCN layer each
core computes its local message rows m = dinv * (h @ W), AllGathers the full
[N,64] message table, then aggregates its own dst nodes by gathering per-edge
rows with dma_gather (4 src%4 residue chunks keep int16 indices in range;
elem_step=256 elems = 4 rows; a zero row at relative idx 25000 serves as pad).
A fixed 4-slots-per-(node,chunk) grid reduces on DVE; overflow edges go
through a per-(window,chunk) one-hot staircase on the PE into PSUM. Mean-pool
partials reduce across cores with one AllReduce; the output MLP is replicated.
"""
import sys
sys.path.insert(0, "/opt/trn_rl_repo")
import numpy as np

N = 100000
E = 1600000
F = 128
C = 64
NH = 256
NOUT = 128
L = 4            # hidden layers (total GCN layers = L + 1)
G = 512
W = 8            # cores
NL = N // W      # 12500 local nodes
P = 128
NWIN = (NL + P - 1) // P            # 98 windows (last partial: 84 nodes)
NLP = NWIN * P                      # 12544 padded local nodes
KQ = 4                              # grid slots per (node, chunk)
NCHUNK = 4
NLZ = NL + 1                        # 12501 rows per core in table (m + zero row)
VPAD = W * NLZ                      # 100008 table rows
WG = 16                             # windows per grid gather call
NWG = (NWIN + WG - 1) // WG         # 7 call groups (last has 2 windows)

_cache = {}


def _build_host_structures(edge_index, batch):
    src = edge_index[0].astype(np.int64)
    dst = edge_index[1].astype(np.int64)
    deg = np.bincount(dst, minlength=N).astype(np.float32) + 1.0

    counts_g = np.bincount(batch, minlength=G).astype(np.float32)

    per_core = []
    # first pass: per-core sorted edge structures + ovf counts per (w, q)
    ovf_counts = np.zeros((W, NWIN, NCHUNK), np.int64)
    for c in range(W):
        lo, hi = c * NL, (c + 1) * NL
        m = (dst >= lo) & (dst < hi)
        es, ed = src[m], dst[m] - lo
        row = es + es // NL                  # table row of src
        q = (row % 4).astype(np.int64)
        ridx = row // 4                      # relative idx within chunk
        node = ed
        # rank within (node, q) group, stable in original order
        key = node * 4 + q
        order = np.argsort(key, kind="stable")
        ks, kd, kq, kk = ridx[order], node[order], q[order], key[order]
        # cumcount within group
        uniq, first_idx = np.unique(kk, return_index=True)
        grp_start = np.zeros(len(kk), np.int64)
        grp_start[first_idx] = first_idx
        grp_start = np.maximum.accumulate(grp_start)
        rank = np.arange(len(kk)) - grp_start
        per_core.append((ks, kd, kq, rank))
        ovf = rank >= KQ
        if ovf.any():
            w_idx = kd[ovf] // P
            np.add.at(ovf_counts[c], (w_idx, kq[ovf]), 1)

    # unify overflow tile counts across cores (SPMD: same program everywhere)
    T_wq = np.ceil(ovf_counts.max(axis=0) / P).astype(np.int64)  # [NWIN, NCHUNK]

    # per-(q, wg) grid call sizes
    grid_call_n = []
    for qq in range(NCHUNK):
        row = []
        for wg in range(NWG):
            nw = min(WG, NWIN - wg * WG)
            row.append(nw * KQ * P)
        grid_call_n.append(row)

    # ovf call layout: per (q, wg): tiles for windows in group, concatenated
    ovf_call_tiles = [[int(T_wq[wg * WG:min(NWIN, (wg + 1) * WG), qq].sum())
                       for wg in range(NWG)] for qq in range(NCHUNK)]
    n_ovf_tiles = int(T_wq.sum())

    def wrap16(a):
        # idx list order j -> [16, n/16] wrapped, replicated to 128 partitions
        n = len(a)
        w = a.reshape(n // 16, 16).T
        return np.tile(w, (8, 1))

    inputs_per_core = []
    for c in range(W):
        ks, kd, kq, rank = per_core[c]
        # ---- grid idx: [q][wg] arrays, slot j = wrel*KQ*P + k*P + p
        grid = np.zeros((NCHUNK, NWIN, KQ, P), np.int16)
        for qq in range(NCHUNK):
            grid[qq] = 3125 * (qq + 1)       # per-chunk zero-row relative idx
        gm = rank < KQ
        grid[kq[gm], kd[gm] // P, rank[gm], kd[gm] % P] = ks[gm].astype(np.int16)
        grid_idx = []
        for qq in range(NCHUNK):
            cols = []
            for wg in range(NWG):
                nw = min(WG, NWIN - wg * WG)
                block = grid[qq, wg * WG:wg * WG + nw]        # [nw, KQ, P]
                cols.append(wrap16(block.reshape(-1)))
            grid_idx.append(np.concatenate(cols, axis=1))
        grid_idx = np.concatenate(grid_idx, axis=1)           # [128, 200k/16]

        # ---- ovf: per (q, w): list of (src//4, dst_rel) padded to T_wq*P
        ovf_idx_parts = []
        ovf_rel_cols = []
        om = rank >= KQ
        os_, od_, oq_ = ks[om], kd[om], kq[om]
        for qq in range(NCHUNK):
            for wg in range(NWG):
                for w_ in range(wg * WG, min(NWIN, (wg + 1) * WG)):
                    t = int(T_wq[w_, qq])
                    if t == 0:
                        continue
                    sel = (oq_ == qq) & (od_ // P == w_)
                    s_sel = os_[sel].astype(np.int16)
                    r_sel = (od_[sel] % P).astype(np.float32)
                    nslot = t * P
                    idx_pad = np.full(nslot, 3125 * (qq + 1), np.int16)
                    rel_pad = np.full(nslot, -1.0, np.float32)
                    idx_pad[:len(s_sel)] = s_sel
                    rel_pad[:len(r_sel)] = r_sel
                    ovf_idx_parts.append(wrap16(idx_pad))
                    ovf_rel_cols.append(rel_pad.reshape(t, P).T)  # [P, t]
        if ovf_idx_parts:
            ovf_idx = np.concatenate(ovf_idx_parts, axis=1)
            ovf_rel = np.concatenate(ovf_rel_cols, axis=1)
        else:
            ovf_idx = np.zeros((P, 16), np.int16)
            ovf_rel = np.zeros((P, 1), np.float32)

        # ---- misc per-core tensors
        degw = np.ones((P, NWIN), np.float32)
        dv = deg[c * NL:(c + 1) * NL]
        degw_flat = np.ones(NLP, np.float32)
        degw_flat[:NL] = dv
        degw = degw_flat.reshape(NWIN, P).T                    # [p, w]

        bl = batch[c * NL:(c + 1) * NL].astype(np.int64)
        base_g = int(bl[0])
        brel = np.full(NLP, -1.0, np.float32)
        brel[:NL] = (bl - base_g).astype(np.float32)
        assert brel.max() < 128, "graph span per core exceeds 128"
        batch_rel = brel.reshape(NWIN, P).T                    # [p, w]

        pool_off = (base_g + np.arange(P)).astype(np.int32)[:, None]  # [128,1]

        inputs_per_core.append(dict(
            grid_idx=grid_idx.astype(np.int16),
            ovf_idx=ovf_idx.astype(np.int16),
            ovf_rel=ovf_rel.astype(np.float32),
            degw=degw.astype(np.float32),
            batch_rel=batch_rel.astype(np.float32),
            pool_off=pool_off,
        ))

    shared = dict(
        grid_call_n=grid_call_n,
        ovf_call_tiles=ovf_call_tiles,
        T_wq=T_wq,
        n_ovf_tiles=n_ovf_tiles,
        counts_g=counts_g,
    )
    return inputs_per_core, shared


def _build_program(shared):
    import concourse.bass as bass
    import concourse.bacc as bacc
    import concourse.mybir as mybir
    import concourse.tile as tile

    grid_call_n = shared["grid_call_n"]
    ovf_call_tiles = shared["ovf_call_tiles"]
    T_wq = shared["T_wq"]
    n_ovf_tiles = max(1, shared["n_ovf_tiles"])

    fp32 = mybir.dt.float32
    nc = bacc.Bacc("TRN2", target_bir_lowering=False, debug=False, num_devices=W)

    x_in = nc.dram_tensor("x", [NLP, F], fp32, kind="ExternalInput")
    W0_in = nc.dram_tensor("w0", [F, C], fp32, kind="ExternalInput")
    Wh_in = nc.dram_tensor("wh", [C, L * C], fp32, kind="ExternalInput")
    Bb_in = nc.dram_tensor("bb", [P, (L + 1) * C], fp32, kind="ExternalInput")
    W1_in = nc.dram_tensor("w1", [C, NH], fp32, kind="ExternalInput")
    b1_in = nc.dram_tensor("b1", [P, 2], fp32, kind="ExternalInput")
    W2_in = nc.dram_tensor("w2", [P, 2 * NOUT], fp32, kind="ExternalInput")
    b2_in = nc.dram_tensor("b2", [NOUT, 1], fp32, kind="ExternalInput")
    gidx_in = nc.dram_tensor("gidx", [P, NCHUNK * NWIN * KQ * P // 16], mybir.dt.int16,
                             kind="ExternalInput")
    oidx_in = nc.dram_tensor("oidx", [P, n_ovf_tiles * P // 16], mybir.dt.int16,
                             kind="ExternalInput")
    orel_in = nc.dram_tensor("orel", [P, n_ovf_tiles], fp32, kind="ExternalInput")
    degw_in = nc.dram_tensor("degw", [P, NWIN], fp32, kind="ExternalInput")
    brel_in = nc.dram_tensor("brel", [P, NWIN], fp32, kind="ExternalInput")
    poff_in = nc.dram_tensor("poff", [P, 1], mybir.dt.int32, kind="ExternalInput")
    cnts_in = nc.dram_tensor("cnts", [P, G // P], fp32, kind="ExternalInput")
    iota_in = nc.dram_tensor("iota", [P, P], fp32, kind="ExternalInput")
    out_t = nc.dram_tensor("out_t", [NOUT, G], fp32, kind="ExternalOutput")

    with tile.TileContext(nc, num_cores=W) as tc:
        with (
            tc.tile_pool(name="const", bufs=1) as constp,
            tc.tile_pool(name="state", bufs=1) as statep,
            tc.tile_pool(name="work", bufs=3) as workp,
            tc.tile_pool(name="gat", bufs=2) as gatp,
            tc.tile_pool(name="ogat", bufs=2) as ogatp,
            tc.tile_pool(name="ps", bufs=3, space="PSUM") as psp,
            tc.tile_pool(name="psw", bufs=2, space="PSUM") as pswp,
            tc.tile_pool(name="dram", bufs=1, space="DRAM") as dramp,
        ):
            # ---------- constants / state ----------
            gidx = constp.tile([P, NCHUNK * NWIN * KQ * P // 16], mybir.dt.int16)
            nc.sync.dma_start(out=gidx[:], in_=gidx_in[:])
            oidx = constp.tile([P, n_ovf_tiles * P // 16], mybir.dt.int16)
            nc.sync.dma_start(out=oidx[:], in_=oidx_in[:])
            orel = constp.tile([P, n_ovf_tiles], fp32)
            nc.sync.dma_start(out=orel[:], in_=orel_in[:])
            iota = constp.tile([P, P], fp32)
            nc.sync.dma_start(out=iota[:], in_=iota_in[:])
            degw = constp.tile([P, NWIN], fp32)
            nc.sync.dma_start(out=degw[:], in_=degw_in[:])
            dinv = constp.tile([P, NWIN], fp32)
            drec = constp.tile([P, NWIN], fp32)
            nc.vector.reciprocal(drec[:], degw[:])
            nc.scalar.activation(dinv[:], drec[:],
                                 mybir.ActivationFunctionType.Sqrt)
            bb = constp.tile([P, (L + 1) * C], fp32)
            nc.sync.dma_start(out=bb[:], in_=Bb_in[:])
            w0 = constp.tile([F, C], fp32)
            nc.sync.dma_start(out=w0[:], in_=W0_in[:])
            wh = constp.tile([C, L * C], fp32)
            nc.sync.dma_start(out=wh[:], in_=Wh_in[:])

            h_sb = statep.tile([P, NWIN * C], fp32)    # current layer activations
            m_sb = statep.tile([P, NWIN * C], fp32)    # message rows (table vals)
            s_sb = statep.tile([P, NWIN * C], fp32)    # aggregation accumulator

            tables = [dramp.tile([VPAD, C], fp32, addr_space="Shared",
                                 name=f"table{li}")
                      for li in range(L + 1)]
            ag_in = dramp.tile([NLZ, C], fp32)
            ztile = constp.tile([1, C], fp32)
            nc.vector.memset(ztile[:], 0.0)
            nc.sync.dma_start(out=ag_in[NL:NLZ, :], in_=ztile[:])

            ident = constp.tile([P, P], fp32)
            from concourse.masks import make_identity
            make_identity(nc, ident[:])

            def transpose_to(dst_sb, src_ap, pn, fn):
                """PE transpose src [pn, fn] -> dst sbuf [fn, pn]."""
                pt = psp.tile([P, P], fp32, tag="ps")
                nc.tensor.transpose(out=pt[:fn, :pn], in_=src_ap, identity=ident[:])
                nc.any.tensor_copy(out=dst_sb, in_=pt[:fn, :pn])

            # ---------- per layer ----------
            for layer in range(L + 1):
                cin = F if layer == 0 else C
                # Phase A: m = dinv * (h @ W); write table; allgather
                for wg4 in range((NWIN + 3) // 4):
                    w0_ = wg4 * 4
                    nw = min(4, NWIN - w0_)
                    hT = workp.tile([cin, 4 * P], fp32, tag="hT")
                    for j in range(nw):
                        w_ = w0_ + j
                        if layer == 0:
                            xw = workp.tile([P, F], fp32, tag="xw")
                            nc.sync.dma_start(
                                out=xw[:],
                                in_=x_in[w_ * P:(w_ + 1) * P, :])
                            transpose_to(hT[:, j * P:(j + 1) * P], xw[:], P, cin)
                        else:
                            transpose_to(hT[:, j * P:(j + 1) * P],
                                         h_sb[:, w_ * C:(w_ + 1) * C], P, cin)
                    mT_ps = psp.tile([C, 4 * P], fp32, tag="ps")
                    lhs = w0[:] if layer == 0 else wh[:, (layer - 1) * C:layer * C]
                    nc.tensor.matmul(mT_ps[:, :nw * P], lhs, hT[:, :nw * P],
                                     start=True, stop=True)
                    mT_sb = workp.tile([C, 4 * P], fp32, tag="mTs")
                    nc.any.tensor_copy(out=mT_sb[:, :nw * P], in_=mT_ps[:, :nw * P])
                    for j in range(nw):
                        w_ = w0_ + j
                        mp = psp.tile([P, C], fp32, tag="ps")
                        nc.tensor.transpose(
                            out=mp[:], in_=mT_sb[:, j * P:(j + 1) * P],
                            identity=ident[:C, :C])
                        nc.vector.tensor_scalar_mul(
                            m_sb[:, w_ * C:(w_ + 1) * C], mp[:],
                            dinv[:, w_:w_ + 1])
                nc.sync.dma_start(
                    out=ag_in[0:(NL // P) * P, :].rearrange("(w p) c -> p w c", p=P),
                    in_=m_sb[:].rearrange("p (w c) -> p w c", c=C)[:, :NL // P, :])
                # last partial window (84 rows)
                nc.sync.dma_start(
                    out=ag_in[(NL // P) * P:NL, :],
                    in_=m_sb[:NL - (NL // P) * P,
                             (NL // P) * C:(NL // P + 1) * C])
                nc.gpsimd.collective_compute(
                    "AllGather", mybir.AluOpType.bypass,
                    replica_groups=[list(range(W))],
                    ins=[ag_in.opt()],
                    outs=[tables[layer].opt()],
                )
                chunks = tables[layer][:].rearrange(
                    "(r four) c -> four r c", four=4)

                # Phase B: per-chunk passes accumulate into s_sb, then epilogue
                g_off = 0
                o_off = 0
                orel_col = {}
                col = 0
                for qq in range(NCHUNK):
                    for wg in range(NWG):
                        for w_ in range(wg * WG, min(NWIN, (wg + 1) * WG)):
                            t = int(T_wq[w_, qq])
                            if t:
                                orel_col[(qq, w_)] = (col, t)
                                col += t
                OTMAX = max(1, max(t for r in ovf_call_tiles for t in r))
                for qq in range(NCHUNK):
                    for wg in range(NWG):
                        nw = min(WG, NWIN - wg * WG)
                        nidx = grid_call_n[qq][wg]
                        gt = gatp.tile([P, WG * KQ, C], fp32, tag="gt")
                        nc.gpsimd.dma_gather(
                            gt[:, :nidx // P, :], chunks[qq],
                            gidx[:, g_off // 16:(g_off + nidx) // 16],
                            nidx, nidx, C, elem_step=256, single_packet=False)
                        g_off += nidx
                        nt = ovf_call_tiles[qq][wg]
                        ot = None
                        if nt:
                            onidx = nt * P
                            ot = ogatp.tile([P, OTMAX, C], fp32, tag="ot")
                            nc.gpsimd.dma_gather(
                                ot[:, :nt, :], chunks[qq],
                                oidx[:, o_off // 16:(o_off + onidx) // 16],
                                onidx, onidx, C, elem_step=256,
                                single_packet=False)
                            o_off += onidx
                        tbase = 0
                        for w_ in range(wg * WG, wg * WG + nw):
                            wrel = w_ - wg * WG
                            view = gt[:, wrel * KQ:(wrel + 1) * KQ, :].rearrange(
                                "p k d -> p d k")
                            if qq == 0:
                                nc.vector.tensor_reduce(
                                    out=s_sb[:, w_ * C:(w_ + 1) * C], in_=view,
                                    op=mybir.AluOpType.add,
                                    axis=mybir.AxisListType.X)
                            else:
                                part = workp.tile([P, C], fp32, tag="part")
                                nc.vector.tensor_reduce(
                                    out=part[:], in_=view,
                                    op=mybir.AluOpType.add,
                                    axis=mybir.AxisListType.X)
                                nc.vector.tensor_add(
                                    out=s_sb[:, w_ * C:(w_ + 1) * C],
                                    in0=s_sb[:, w_ * C:(w_ + 1) * C],
                                    in1=part[:])
                            if (qq, w_) in orel_col:
                                col0, t = orel_col[(qq, w_)]
                                op_ = pswp.tile([P, C], fp32, tag="psw")
                                for ti in range(t):
                                    oh = workp.tile([P, P], fp32, tag="oh")
                                    nc.any.tensor_tensor(
                                        out=oh[:],
                                        in0=orel[:, col0 + ti:col0 + ti + 1]
                                            .to_broadcast([P, P]),
                                        in1=iota[:],
                                        op=mybir.AluOpType.is_equal)
                                    nc.tensor.matmul(
                                        op_[:], oh[:], ot[:, tbase + ti, :],
                                        start=(ti == 0), stop=(ti == t - 1))
                                tbase += t
                                nc.vector.tensor_add(
                                    out=s_sb[:, w_ * C:(w_ + 1) * C],
                                    in0=s_sb[:, w_ * C:(w_ + 1) * C],
                                    in1=op_[:])
                for w_ in range(NWIN):
                    sw = workp.tile([P, C], fp32, tag="sw")
                    nc.vector.tensor_add(out=sw[:],
                                         in0=s_sb[:, w_ * C:(w_ + 1) * C],
                                         in1=m_sb[:, w_ * C:(w_ + 1) * C])
                    nc.vector.tensor_scalar_mul(sw[:], sw[:], dinv[:, w_:w_ + 1])
                    nc.vector.tensor_add(out=sw[:], in0=sw[:],
                                         in1=bb[:, layer * C:(layer + 1) * C])
                    nc.scalar.activation(h_sb[:, w_ * C:(w_ + 1) * C], sw[:],
                                         mybir.ActivationFunctionType.Relu)

            # ---------- pooling ----------
            brel = constp.tile([P, NWIN], fp32)
            nc.sync.dma_start(out=brel[:], in_=brel_in[:])
            pool_ps = pswp.tile([C, P], fp32, tag="psw")
            for w_ in range(NWIN):
                ohp = workp.tile([P, P], fp32, tag="ohp")
                nc.any.tensor_tensor(
                    out=ohp[:],
                    in0=brel[:, w_:w_ + 1].to_broadcast([P, P]),
                    in1=iota[:], op=mybir.AluOpType.is_equal)
                nc.tensor.matmul(pool_ps[:], h_sb[:, w_ * C:(w_ + 1) * C],
                                 ohp[:], start=(w_ == 0), stop=(w_ == NWIN - 1))
            poolT = constp.tile([C, P], fp32)
            nc.any.tensor_copy(out=poolT[:], in_=pool_ps[:])
            pool_n = constp.tile([P, C], fp32)
            pp = psp.tile([P, C], fp32, tag="ps")
            nc.tensor.transpose(out=pp[:], in_=poolT[:], identity=ident[:C, :C])
            nc.any.tensor_copy(out=pool_n[:], in_=pp[:])

            ar_in = dramp.tile([640, C], fp32)
            ar_out = dramp.tile([640, C], fp32, addr_space="Shared")
            zt = constp.tile([P, C], fp32)
            nc.vector.memset(zt[:], 0.0)
            for z5 in range(5):
                nc.sync.dma_start(out=ar_in[z5 * P:(z5 + 1) * P, :], in_=zt[:])
            poff = constp.tile([P, 1], mybir.dt.int32)
            nc.sync.dma_start(out=poff[:], in_=poff_in[:])
            nc.gpsimd.indirect_dma_start(
                out=ar_in[:],
                out_offset=bass.IndirectOffsetOnAxis(ap=poff[:, :1], axis=0),
                in_=pool_n[:], in_offset=None)
            nc.gpsimd.collective_compute(
                "AllReduce", mybir.AluOpType.add,
                replica_groups=[list(range(W))],
                ins=[ar_in.opt()], outs=[ar_out.opt()],
            )
            # load pooled sums [512] -> [128, 4, 64]; normalize by counts
            pools = constp.tile([P, G // P, C], fp32)
            nc.sync.dma_start(
                out=pools[:],
                in_=ar_out[0:G, :].rearrange("(w p) c -> p w c", p=P))
            cnts = constp.tile([P, G // P], fp32)
            nc.sync.dma_start(out=cnts[:], in_=cnts_in[:])
            cmax = constp.tile([P, G // P], fp32)
            nc.vector.tensor_scalar_max(cmax[:], cnts[:], 1.0)
            crec = constp.tile([P, G // P], fp32)
            nc.vector.reciprocal(crec[:], cmax[:])
            for j in range(G // P):
                nc.vector.tensor_scalar_mul(
                    pools[:, j, :], pools[:, j, :], crec[:, j:j + 1])
            # transpose pooled -> [64, 512]
            pT_ps = psp.tile([C, G], fp32, tag="ps")
            for j in range(G // P):
                nc.tensor.transpose(out=pT_ps[:, j * P:(j + 1) * P],
                                    in_=pools[:, j, :], identity=ident[:])
            pT = constp.tile([C, G], fp32)
            nc.any.tensor_copy(out=pT[:], in_=pT_ps[:])
            # MLP
            w1t = constp.tile([C, NH], fp32)
            nc.sync.dma_start(out=w1t[:], in_=W1_in[:])
            b1t = constp.tile([P, 2], fp32)
            nc.sync.dma_start(out=b1t[:], in_=b1_in[:])
            w2t = constp.tile([P, 2 * NOUT], fp32)
            nc.sync.dma_start(out=w2t[:], in_=W2_in[:])
            b2t = constp.tile([NOUT, 1], fp32)
            nc.sync.dma_start(out=b2t[:], in_=b2_in[:])

            a1 = constp.tile([P, 2 * G], fp32)
            for half in range(2):
                z1 = pswp.tile([P, G], fp32, tag="psw")
                nc.tensor.matmul(z1[:], w1t[:, half * P:(half + 1) * P], pT[:],
                                 start=True, stop=True)
                nc.scalar.activation(
                    a1[:, half * G:(half + 1) * G], z1[:],
                    mybir.ActivationFunctionType.Relu,
                    bias=b1t[:, half:half + 1])
            z2 = pswp.tile([NOUT, G], fp32, tag="psw")
            nc.tensor.matmul(z2[:], w2t[:, 0:NOUT], a1[:, 0:G],
                             start=True, stop=False)
            nc.tensor.matmul(z2[:], w2t[:, NOUT:2 * NOUT], a1[:, G:2 * G],
                             start=False, stop=True)
            outs = constp.tile([NOUT, G], fp32)
            nc.vector.tensor_scalar_add(outs[:], z2[:], b2t[:, 0:1])
            nc.sync.dma_start(out=out_t[:], in_=outs[:])

    nc.finalize()
    return nc


def kernel(x, edge_index, batch, W0, b0, Wh, bh, W1, b1, W2, b2):
    x = np.asarray(x)
    edge_index = np.asarray(edge_index)
    batch = np.asarray(batch)

    key = "prog"
    if key not in _cache:
        inputs_per_core, shared = _build_host_structures(edge_index, batch)
        nc = _build_program(shared)
        _cache[key] = (nc, inputs_per_core, shared)
    nc, inputs_per_core, shared = _cache[key]

    # bias broadcast tensors [L+1, 128, C]
    bvec = np.concatenate([np.asarray(b0, np.float32)[None, :],
                           np.asarray(bh, np.float32)], axis=0)   # [L+1, C]
    bb = np.tile(bvec.reshape(1, (L + 1) * C), (P, 1))

    iota = np.tile(np.arange(P, dtype=np.float32)[None, :], (P, 1))
    cnts = shared["counts_g"].reshape(G // P, P).T.astype(np.float32)

    in_maps = []
    for c in range(W):
        pc = inputs_per_core[c]
        xl = np.zeros((NLP, F), np.float32)
        xl[:NL] = np.asarray(x, np.float32)[c * NL:(c + 1) * NL]
        in_maps.append(dict(
            x=xl,
            w0=np.asarray(W0, np.float32),
            wh=np.ascontiguousarray(np.asarray(Wh, np.float32).transpose(1, 0, 2).reshape(C, L * C)),
            bb=bb,
            w1=np.asarray(W1, np.float32),
            b1=np.ascontiguousarray(np.asarray(b1, np.float32).reshape(2, P).T),
            w2=np.ascontiguousarray(np.asarray(W2, np.float32).reshape(2, P, NOUT).transpose(1, 0, 2).reshape(P, 2 * NOUT)),
            b2=np.asarray(b2, np.float32)[:, None],
            gidx=pc["grid_idx"],
            oidx=pc["ovf_idx"],
            orel=pc["ovf_rel"],
            degw=pc["degw"],
            brel=pc["batch_rel"],
            poff=pc["pool_off"],
            cnts=cnts,
            iota=iota,
        ))

    from concourse.bass_utils import run_bass_kernel_spmd
    res = run_bass_kernel_spmd(nc, in_maps, list(range(W)))
    out_t = res.results[0]["out_t"]          # [NOUT, G]
    return np.ascontiguousarray(out_t.T.astype(np.float32))


if __name__ == "__main__":
    pass


# revision 27
# speedup vs baseline: 166.7786x; 166.7786x over previous
"""GCN graph encoder on 8 Trainium2 NeuronCores (Bass/Tile SPMD).

Sharding: nodes (and incident edges, by dst) split 8 ways. Per GCN layer each
core computes its local message rows m = dinv * (h @ W), AllGathers the full
[N,64] message table, then aggregates its own dst nodes by gathering per-edge
rows with dma_gather (4 src%4 residue chunks keep int16 indices in range;
elem_step=256 elems = 4 rows; a zero row at relative idx 25000 serves as pad).
A fixed 4-slots-per-(node,chunk) grid reduces on DVE; overflow edges go
through a per-(window,chunk) one-hot staircase on the PE into PSUM. Mean-pool
partials reduce across cores with one AllReduce; the output MLP is replicated.
"""
import sys, os
sys.path.insert(0, "/opt/trn_rl_repo")
import numpy as np
SKIP_OVF = os.environ.get("SKIP_OVF") == "1"
SKIP_GRID = os.environ.get("SKIP_GRID") == "1"
SKIP_AG = os.environ.get("SKIP_AG") == "1"
SKIP_STAIR = os.environ.get("SKIP_STAIR") == "1"

N = 100000
E = 1600000
F = 128
C = 64
NH = 256
NOUT = 128
L = 4            # hidden layers (total GCN layers = L + 1)
G = 512
W = 8            # cores
NL = N // W      # 12500 local nodes
P = 128
NWIN = (NL + P - 1) // P            # 98 windows (last partial: 84 nodes)
NLP = NWIN * P                      # 12544 padded local nodes
KQ = 4                              # grid slots per (node, chunk)
NCHUNK = 4
NLZ = NL + 1                        # 12501 rows per core in table (m + zero row)
VPAD = W * NLZ                      # 100008 table rows
WG = 14                             # windows per grid gather call
NWG = (NWIN + WG - 1) // WG         # 7 call groups (last has 2 windows)

_cache = {}


def _build_host_structures(edge_index, batch):
    src = edge_index[0].astype(np.int64)
    dst = edge_index[1].astype(np.int64)
    deg = np.bincount(dst, minlength=N).astype(np.float32) + 1.0

    counts_g = np.bincount(batch, minlength=G).astype(np.float32)

    per_core = []
    # first pass: per-core sorted edge structures + ovf counts per (w, q)
    ovf_counts = np.zeros((W, NWIN, NCHUNK), np.int64)
    for c in range(W):
        lo, hi = c * NL, (c + 1) * NL
        m = (dst >= lo) & (dst < hi)
        es, ed = src[m], dst[m] - lo
        row = es + es // NL                  # table row of src
        q = (row % 4).astype(np.int64)
        ridx = row // 4                      # relative idx within chunk
        node = ed
        # rank within (node, q) group, stable in original order
        key = node * 4 + q
        order = np.argsort(key, kind="stable")
        ks, kd, kq, kk = ridx[order], node[order], q[order], key[order]
        # cumcount within group
        uniq, first_idx = np.unique(kk, return_index=True)
        grp_start = np.zeros(len(kk), np.int64)
        grp_start[first_idx] = first_idx
        grp_start = np.maximum.accumulate(grp_start)
        rank = np.arange(len(kk)) - grp_start
        per_core.append((ks, kd, kq, rank))
        ovf = rank >= KQ
        if ovf.any():
            w_idx = kd[ovf] // P
            np.add.at(ovf_counts[c], (w_idx, kq[ovf]), 1)

    # unify overflow tile counts across cores (SPMD: same program everywhere)
    T_wq = np.ceil(ovf_counts.max(axis=0) / P).astype(np.int64)  # [NWIN, NCHUNK]

    # per-(q, wg) grid call sizes
    grid_call_n = []
    for qq in range(NCHUNK):
        row = []
        for wg in range(NWG):
            nw = min(WG, NWIN - wg * WG)
            row.append(nw * KQ * P)
        grid_call_n.append(row)

    # ovf call layout: per (q, wg): tiles for windows in group, concatenated
    ovf_call_tiles = [[int(T_wq[wg * WG:min(NWIN, (wg + 1) * WG), qq].sum())
                       for wg in range(NWG)] for qq in range(NCHUNK)]
    n_ovf_tiles = int(T_wq.sum())

    def wrap16(a):
        # idx list order j -> [16, n/16] wrapped, replicated to 128 partitions
        n = len(a)
        w = a.reshape(n // 16, 16).T
        return np.tile(w, (8, 1))

    inputs_per_core = []
    for c in range(W):
        ks, kd, kq, rank = per_core[c]
        # ---- grid idx: [q][wg] arrays, slot j = wrel*KQ*P + k*P + p
        grid = np.zeros((NCHUNK, NWIN, KQ, P), np.int16)
        for qq in range(NCHUNK):
            grid[qq] = 3125 * (qq + 1)       # per-chunk zero-row relative idx
        gm = rank < KQ
        grid[kq[gm], kd[gm] // P, rank[gm], kd[gm] % P] = ks[gm].astype(np.int16)
        grid_idx = []
        for qq in range(NCHUNK):
            cols = []
            for wg in range(NWG):
                nw = min(WG, NWIN - wg * WG)
                block = grid[qq, wg * WG:wg * WG + nw]        # [nw, KQ, P]
                cols.append(wrap16(block.reshape(-1)))
            grid_idx.append(np.concatenate(cols, axis=1))
        grid_idx = np.concatenate(grid_idx, axis=1)           # [128, 200k/16]

        # ---- ovf: per (q, w): list of (src//4, dst_rel) padded to T_wq*P
        ovf_idx_parts = []
        ovf_rel_cols = []
        om = rank >= KQ
        os_, od_, oq_ = ks[om], kd[om], kq[om]
        for qq in range(NCHUNK):
            for wg in range(NWG):
                for w_ in range(wg * WG, min(NWIN, (wg + 1) * WG)):
                    t = int(T_wq[w_, qq])
                    if t == 0:
                        continue
                    sel = (oq_ == qq) & (od_ // P == w_)
                    s_sel = os_[sel].astype(np.int16)
                    r_sel = (od_[sel] % P).astype(np.float32)
                    nslot = t * P
                    idx_pad = np.full(nslot, 3125 * (qq + 1), np.int16)
                    rel_pad = np.full(nslot, -1.0, np.float32)
                    idx_pad[:len(s_sel)] = s_sel
                    rel_pad[:len(r_sel)] = r_sel
                    ovf_idx_parts.append(wrap16(idx_pad))
                    ovf_rel_cols.append(rel_pad.reshape(t, P).T)  # [P, t]
        if ovf_idx_parts:
            ovf_idx = np.concatenate(ovf_idx_parts, axis=1)
            ovf_rel = np.concatenate(ovf_rel_cols, axis=1)
        else:
            ovf_idx = np.zeros((P, 16), np.int16)
            ovf_rel = np.zeros((P, 1), np.float32)

        # ---- misc per-core tensors
        degw = np.ones((P, NWIN), np.float32)
        dv = deg[c * NL:(c + 1) * NL]
        degw_flat = np.ones(NLP, np.float32)
        degw_flat[:NL] = dv
        degw = degw_flat.reshape(NWIN, P).T                    # [p, w]

        bl = batch[c * NL:(c + 1) * NL].astype(np.int64)
        base_g = int(bl[0])
        brel = np.full(NLP, -1.0, np.float32)
        brel[:NL] = (bl - base_g).astype(np.float32)
        assert brel.max() < 128, "graph span per core exceeds 128"
        batch_rel = brel.reshape(NWIN, P).T                    # [p, w]

        pool_off = (base_g + np.arange(P)).astype(np.int32)[:, None]  # [128,1]

        inputs_per_core.append(dict(
            grid_idx=grid_idx.astype(np.int16),
            ovf_idx=ovf_idx.astype(np.int16),
            ovf_rel=ovf_rel.astype(np.float32),
            degw=degw.astype(np.float32),
            batch_rel=batch_rel.astype(np.float32),
            pool_off=pool_off,
        ))

    shared = dict(
        grid_call_n=grid_call_n,
        ovf_call_tiles=ovf_call_tiles,
        T_wq=T_wq,
        n_ovf_tiles=n_ovf_tiles,
        counts_g=counts_g,
    )
    return inputs_per_core, shared


def _build_program(shared, repeat=None):
    import concourse.bass as bass
    import concourse.bacc as bacc
    import concourse.mybir as mybir
    import concourse.tile as tile

    REPEAT = int(os.environ.get("REPEAT", "1")) if repeat is None else repeat
    grid_call_n = shared["grid_call_n"]
    ovf_call_tiles = shared["ovf_call_tiles"]
    T_wq = shared["T_wq"]
    n_ovf_tiles = max(1, shared["n_ovf_tiles"])

    fp32 = mybir.dt.float32
    nc = bacc.Bacc("TRN2", target_bir_lowering=False, debug=False, num_devices=W)

    x_in = nc.dram_tensor("x", [NLP, F], fp32, kind="ExternalInput")
    W0_in = nc.dram_tensor("w0", [F, C], fp32, kind="ExternalInput")
    Wh_in = nc.dram_tensor("wh", [C, L * C], fp32, kind="ExternalInput")
    Bb_in = nc.dram_tensor("bb", [P, (L + 1) * C], fp32, kind="ExternalInput")
    W1_in = nc.dram_tensor("w1", [C, NH], fp32, kind="ExternalInput")
    b1_in = nc.dram_tensor("b1", [P, 2], fp32, kind="ExternalInput")
    W2_in = nc.dram_tensor("w2", [P, 2 * NOUT], fp32, kind="ExternalInput")
    b2_in = nc.dram_tensor("b2", [NOUT, 1], fp32, kind="ExternalInput")
    gidx_in = nc.dram_tensor("gidx", [P, NCHUNK * NWIN * KQ * P // 16], mybir.dt.int16,
                             kind="ExternalInput")
    oidx_in = nc.dram_tensor("oidx", [P, n_ovf_tiles * P // 16], mybir.dt.int16,
                             kind="ExternalInput")
    orel_in = nc.dram_tensor("orel", [P, n_ovf_tiles], fp32, kind="ExternalInput")
    degw_in = nc.dram_tensor("degw", [P, NWIN], fp32, kind="ExternalInput")
    brel_in = nc.dram_tensor("brel", [P, NWIN], fp32, kind="ExternalInput")
    poff_in = nc.dram_tensor("poff", [P, 1], mybir.dt.int32, kind="ExternalInput")
    cnts_in = nc.dram_tensor("cnts", [P, G // P], fp32, kind="ExternalInput")
    iota_in = nc.dram_tensor("iota", [P, P], fp32, kind="ExternalInput")
    out_t = nc.dram_tensor("out_t", [NOUT, G], fp32, kind="ExternalOutput")

    with tile.TileContext(nc, num_cores=W) as tc:
        with (
            tc.tile_pool(name="const", bufs=1) as constp,
            tc.tile_pool(name="state", bufs=1) as statep,
            tc.tile_pool(name="work", bufs=3) as workp,
            tc.tile_pool(name="gat", bufs=3) as gatp,
            tc.tile_pool(name="ogat", bufs=4) as ogatp,
            tc.tile_pool(name="ps", bufs=4, space="PSUM") as psp,
            tc.tile_pool(name="psw", bufs=3, space="PSUM") as pswp,
            tc.tile_pool(name="dram", bufs=1, space="DRAM") as dramp,
        ):
            # ---------- constants / state ----------
            gidx = constp.tile([P, NCHUNK * NWIN * KQ * P // 16], mybir.dt.int16)
            nc.sync.dma_start(out=gidx[:], in_=gidx_in[:])
            oidx = constp.tile([P, n_ovf_tiles * P // 16], mybir.dt.int16)
            nc.sync.dma_start(out=oidx[:], in_=oidx_in[:])
            orel = constp.tile([P, n_ovf_tiles], fp32)
            nc.sync.dma_start(out=orel[:], in_=orel_in[:])
            iota = constp.tile([P, P], fp32)
            nc.sync.dma_start(out=iota[:], in_=iota_in[:])
            degw = constp.tile([P, NWIN], fp32)
            nc.sync.dma_start(out=degw[:], in_=degw_in[:])
            dinv = constp.tile([P, NWIN], fp32)
            nc.vector.reciprocal(dinv[:], degw[:])
            nc.scalar.activation(dinv[:], dinv[:],
                                 mybir.ActivationFunctionType.Sqrt)
            bb = constp.tile([P, (L + 1) * C], fp32)
            nc.sync.dma_start(out=bb[:], in_=Bb_in[:])
            w0 = constp.tile([F, C], fp32)
            nc.sync.dma_start(out=w0[:], in_=W0_in[:])
            wh = constp.tile([C, L * C], fp32)
            nc.sync.dma_start(out=wh[:], in_=Wh_in[:])

            h_w = [statep.tile([P, C], fp32, name=f"h{w}") for w in range(NWIN)]
            m_w = [statep.tile([P, C], fp32, name=f"m{w}") for w in range(NWIN)]
            s_w = [statep.tile([P, C], fp32, name=f"s{w}") for w in range(NWIN)]

            tables = [dramp.tile([VPAD, C], fp32, addr_space="Shared",
                                 name=f"table{li}")
                      for li in range(L + 1)]
            ag_in = dramp.tile([NLZ, C], fp32)
            ztile = constp.tile([1, C], fp32)
            nc.vector.memset(ztile[:], 0.0)
            nc.sync.dma_start(out=ag_in[NL:NLZ, :], in_=ztile[:])

            ident = constp.tile([P, P], fp32)
            from concourse.masks import make_identity
            make_identity(nc, ident[:])

            def transpose_to(dst_sb, src_ap, pn, fn):
                """PE transpose src [pn, fn] -> dst sbuf [fn, pn]."""
                pt = psp.tile([P, P], fp32, tag="ps")
                nc.tensor.transpose(out=pt[:fn, :pn], in_=src_ap, identity=ident[:])
                nc.any.tensor_copy(out=dst_sb, in_=pt[:fn, :pn])

            # ---------- per layer ----------
            for layer in range(L + 1):
                cin = F if layer == 0 else C
                # Phase A: m = dinv * (h @ W); write table; allgather
                for wg4 in range((NWIN + 3) // 4):
                    w0_ = wg4 * 4
                    nw = min(4, NWIN - w0_)
                    hT = workp.tile([cin, 4 * P], fp32, tag="hT")
                    for j in range(nw):
                        w_ = w0_ + j
                        if layer == 0:
                            xw = workp.tile([P, F], fp32, tag="xw")
                            nc.sync.dma_start(
                                out=xw[:],
                                in_=x_in[w_ * P:(w_ + 1) * P, :])
                            transpose_to(hT[:, j * P:(j + 1) * P], xw[:], P, cin)
                        else:
                            transpose_to(hT[:, j * P:(j + 1) * P],
                                         h_w[w_][:], P, cin)
                    mT_ps = psp.tile([C, 4 * P], fp32, tag="ps")
                    lhs = w0[:] if layer == 0 else wh[:, (layer - 1) * C:layer * C]
                    nc.tensor.matmul(mT_ps[:, :nw * P], lhs, hT[:, :nw * P],
                                     start=True, stop=True)
                    mT_sb = workp.tile([C, 4 * P], fp32, tag="mTs")
                    nc.any.tensor_copy(out=mT_sb[:, :nw * P], in_=mT_ps[:, :nw * P])
                    for j in range(nw):
                        w_ = w0_ + j
                        mp = psp.tile([P, C], fp32, tag="ps")
                        nc.tensor.transpose(
                            out=mp[:], in_=mT_sb[:, j * P:(j + 1) * P],
                            identity=ident[:C, :C])
                        nc.vector.tensor_scalar_mul(
                            m_w[w_][:], mp[:],
                            dinv[:, w_:w_ + 1])
                for w_ in range(NWIN - 1):
                    nc.sync.dma_start(out=ag_in[w_ * P:(w_ + 1) * P, :],
                                      in_=m_w[w_][:])
                nc.sync.dma_start(
                    out=ag_in[(NWIN - 1) * P:NL, :],
                    in_=m_w[NWIN - 1][:NL - (NWIN - 1) * P, :])
                if not SKIP_AG:
                    nc.gpsimd.collective_compute(
                        "AllGather", mybir.AluOpType.bypass,
                        replica_groups=[list(range(W))],
                        ins=[ag_in.opt()],
                        outs=[tables[layer].opt()],
                    )
                chunks = tables[layer][:].rearrange(
                    "(r four) c -> four r c", four=4)

                # Phase B: per-chunk passes accumulate into s_sb, then epilogue
                g_off = 0
                o_off = 0
                orel_col = {}
                col = 0
                for qq in range(NCHUNK):
                    for wg in range(NWG):
                        for w_ in range(wg * WG, min(NWIN, (wg + 1) * WG)):
                            t = int(T_wq[w_, qq])
                            if t:
                                orel_col[(qq, w_)] = (col, t)
                                col += t
                OTMAX = max(1, max(t for r in ovf_call_tiles for t in r))
                for qq in range(NCHUNK):
                    for wg in range(NWG):
                        nw = min(WG, NWIN - wg * WG)
                        nidx = grid_call_n[qq][wg]
                        gt = gatp.tile([P, WG * KQ, C], fp32, tag="gt")
                        if not SKIP_GRID:
                            nc.gpsimd.dma_gather(
                                gt[:, :nidx // P, :], chunks[qq],
                                gidx[:, g_off // 16:(g_off + nidx) // 16],
                                nidx, nidx, C, elem_step=256, single_packet=False)
                        g_off += nidx
                        nt = 0 if SKIP_OVF else ovf_call_tiles[qq][wg]
                        ot = None
                        if nt:
                            onidx = nt * P
                            ot = ogatp.tile([P, OTMAX, C], fp32, tag="ot")
                            nc.gpsimd.dma_gather(
                                ot[:, :nt, :], chunks[qq],
                                oidx[:, o_off // 16:(o_off + onidx) // 16],
                                onidx, onidx, C, elem_step=256,
                                single_packet=False)
                            o_off += onidx
                        tbase = 0
                        for w_ in range(wg * WG, wg * WG + nw):
                            wrel = w_ - wg * WG
                            view = gt[:, wrel * KQ:(wrel + 1) * KQ, :].rearrange(
                                "p k d -> p d k")
                            if qq == 0:
                                nc.vector.tensor_reduce(
                                    out=s_w[w_][:], in_=view,
                                    op=mybir.AluOpType.add,
                                    axis=mybir.AxisListType.X)
                            else:
                                part = workp.tile([P, C], fp32, tag="part")
                                nc.vector.tensor_reduce(
                                    out=part[:], in_=view,
                                    op=mybir.AluOpType.add,
                                    axis=mybir.AxisListType.X)
                                nc.vector.tensor_add(
                                    out=s_w[w_][:],
                                    in0=s_w[w_][:],
                                    in1=part[:])
                            if not (SKIP_OVF or SKIP_STAIR) and (qq, w_) in orel_col:
                                col0, t = orel_col[(qq, w_)]
                                op_ = pswp.tile([P, C], fp32, tag="psw")
                                for ti in range(t):
                                    oh = workp.tile([P, P], fp32, tag="oh")
                                    nc.any.tensor_tensor(
                                        out=oh[:],
                                        in0=orel[:, col0 + ti:col0 + ti + 1]
                                            .to_broadcast([P, P]),
                                        in1=iota[:],
                                        op=mybir.AluOpType.is_equal)
                                    nc.tensor.matmul(
                                        op_[:], oh[:], ot[:, tbase + ti, :],
                                        start=(ti == 0), stop=(ti == t - 1))
                                tbase += t
                                nc.vector.tensor_add(
                                    out=s_w[w_][:],
                                    in0=s_w[w_][:],
                                    in1=op_[:])
                for w_ in range(NWIN):
                    sw = workp.tile([P, C], fp32, tag="sw")
                    nc.vector.tensor_add(out=sw[:],
                                         in0=s_w[w_][:],
                                         in1=m_w[w_][:])
                    nc.vector.tensor_scalar_mul(sw[:], sw[:], dinv[:, w_:w_ + 1])
                    nc.vector.tensor_add(out=sw[:], in0=sw[:],
                                         in1=bb[:, layer * C:(layer + 1) * C])
                    nc.scalar.activation(h_w[w_][:], sw[:],
                                         mybir.ActivationFunctionType.Relu)

            # ---------- pooling ----------
            brel = constp.tile([P, NWIN], fp32)
            nc.sync.dma_start(out=brel[:], in_=brel_in[:])
            pool_ps = pswp.tile([C, P], fp32, tag="psw")
            for w_ in range(NWIN):
                ohp = workp.tile([P, P], fp32, tag="ohp")
                nc.any.tensor_tensor(
                    out=ohp[:],
                    in0=brel[:, w_:w_ + 1].to_broadcast([P, P]),
                    in1=iota[:], op=mybir.AluOpType.is_equal)
                nc.tensor.matmul(pool_ps[:], h_w[w_][:],
                                 ohp[:], start=(w_ == 0), stop=(w_ == NWIN - 1))
            poolT = constp.tile([C, P], fp32)
            nc.any.tensor_copy(out=poolT[:], in_=pool_ps[:])
            pool_n = constp.tile([P, C], fp32)
            pp = psp.tile([P, C], fp32, tag="ps")
            nc.tensor.transpose(out=pp[:], in_=poolT[:], identity=ident[:C, :C])
            nc.any.tensor_copy(out=pool_n[:], in_=pp[:])

            ar_in = dramp.tile([640, C], fp32)
            ar_out = dramp.tile([640, C], fp32, addr_space="Shared")
            zt = constp.tile([P, C], fp32)
            nc.vector.memset(zt[:], 0.0)
            for z5 in range(5):
                nc.sync.dma_start(out=ar_in[z5 * P:(z5 + 1) * P, :], in_=zt[:])
            poff = constp.tile([P, 1], mybir.dt.int32)
            nc.sync.dma_start(out=poff[:], in_=poff_in[:])
            nc.gpsimd.indirect_dma_start(
                out=ar_in[:],
                out_offset=bass.IndirectOffsetOnAxis(ap=poff[:, :1], axis=0),
                in_=pool_n[:], in_offset=None)
            nc.gpsimd.collective_compute(
                "AllReduce", mybir.AluOpType.add,
                replica_groups=[list(range(W))],
                ins=[ar_in.opt()], outs=[ar_out.opt()],
            )
            # load pooled sums [512] -> [128, 4, 64]; normalize by counts
            pools = constp.tile([P, G // P, C], fp32)
            nc.sync.dma_start(
                out=pools[:],
                in_=ar_out[0:G, :].rearrange("(w p) c -> p w c", p=P))
            cnts = constp.tile([P, G // P], fp32)
            nc.sync.dma_start(out=cnts[:], in_=cnts_in[:])
            cmax = constp.tile([P, G // P], fp32)
            nc.vector.tensor_scalar_max(cmax[:], cnts[:], 1.0)
            crec = constp.tile([P, G // P], fp32)
            nc.vector.reciprocal(crec[:], cmax[:])
            for j in range(G // P):
                nc.vector.tensor_scalar_mul(
                    pools[:, j, :], pools[:, j, :], crec[:, j:j + 1])
            # transpose pooled -> [64, 512]
            pT_ps = psp.tile([C, G], fp32, tag="ps")
            for j in range(G // P):
                nc.tensor.transpose(out=pT_ps[:, j * P:(j + 1) * P],
                                    in_=pools[:, j, :], identity=ident[:])
            pT = constp.tile([C, G], fp32)
            nc.any.tensor_copy(out=pT[:], in_=pT_ps[:])
            # MLP
            w1t = constp.tile([C, NH], fp32)
            nc.sync.dma_start(out=w1t[:], in_=W1_in[:])
            b1t = constp.tile([P, 2], fp32)
            nc.sync.dma_start(out=b1t[:], in_=b1_in[:])
            w2t = constp.tile([P, 2 * NOUT], fp32)
            nc.sync.dma_start(out=w2t[:], in_=W2_in[:])
            b2t = constp.tile([NOUT, 1], fp32)
            nc.sync.dma_start(out=b2t[:], in_=b2_in[:])

            a1 = constp.tile([P, 2 * G], fp32)
            for half in range(2):
                z1 = pswp.tile([P, G], fp32, tag="psw")
                nc.tensor.matmul(z1[:], w1t[:, half * P:(half + 1) * P], pT[:],
                                 start=True, stop=True)
                nc.scalar.activation(
                    a1[:, half * G:(half + 1) * G], z1[:],
                    mybir.ActivationFunctionType.Relu,
                    bias=b1t[:, half:half + 1])
            z2 = pswp.tile([NOUT, G], fp32, tag="psw")
            nc.tensor.matmul(z2[:], w2t[:, 0:NOUT], a1[:, 0:G],
                             start=True, stop=False)
            nc.tensor.matmul(z2[:], w2t[:, NOUT:2 * NOUT], a1[:, G:2 * G],
                             start=False, stop=True)
            outs = constp.tile([NOUT, G], fp32)
            nc.vector.tensor_scalar_add(outs[:], z2[:], b2t[:, 0:1])
            nc.sync.dma_start(out=out_t[:], in_=outs[:])

    nc.finalize()
    return nc


def kernel(x, edge_index, batch, W0, b0, Wh, bh, W1, b1, W2, b2):
    x = np.asarray(x)
    edge_index = np.asarray(edge_index)
    batch = np.asarray(batch)

    key = "prog"
    if key not in _cache:
        inputs_per_core, shared = _build_host_structures(edge_index, batch)
        nc = _build_program(shared)
        _cache[key] = (nc, inputs_per_core, shared)
    nc, inputs_per_core, shared = _cache[key]

    # bias broadcast tensors [L+1, 128, C]
    bvec = np.concatenate([np.asarray(b0, np.float32)[None, :],
                           np.asarray(bh, np.float32)], axis=0)   # [L+1, C]
    bb = np.tile(bvec.reshape(1, (L + 1) * C), (P, 1))

    iota = np.tile(np.arange(P, dtype=np.float32)[None, :], (P, 1))
    cnts = shared["counts_g"].reshape(G // P, P).T.astype(np.float32)

    in_maps = []
    for c in range(W):
        pc = inputs_per_core[c]
        xl = np.zeros((NLP, F), np.float32)
        xl[:NL] = np.asarray(x, np.float32)[c * NL:(c + 1) * NL]
        in_maps.append(dict(
            x=xl,
            w0=np.asarray(W0, np.float32),
            wh=np.ascontiguousarray(np.asarray(Wh, np.float32).transpose(1, 0, 2).reshape(C, L * C)),
            bb=bb,
            w1=np.asarray(W1, np.float32),
            b1=np.ascontiguousarray(np.asarray(b1, np.float32).reshape(2, P).T),
            w2=np.ascontiguousarray(np.asarray(W2, np.float32).reshape(2, P, NOUT).transpose(1, 0, 2).reshape(P, 2 * NOUT)),
            b2=np.asarray(b2, np.float32)[:, None],
            gidx=pc["grid_idx"],
            oidx=pc["ovf_idx"],
            orel=pc["ovf_rel"],
            degw=pc["degw"],
            brel=pc["batch_rel"],
            poff=pc["pool_off"],
            cnts=cnts,
            iota=iota,
        ))

    from concourse.bass_utils import run_bass_kernel_spmd
    res = run_bass_kernel_spmd(nc, in_maps, list(range(W)))
    out_t = res.results[0]["out_t"]          # [NOUT, G]
    return np.ascontiguousarray(out_t.T.astype(np.float32))


if __name__ == "__main__":
    pass
